# revision 27
# baseline (speedup 1.0000x reference)
"""Trainium2 Bass kernel for nn_Codec (5-level lifting wavelet codec stats).

kernel(**inputs) takes the FULL inputs (x [32,3,512,512] f32 + eight 3-tap
filters) and returns (loss1, loss0, invCR0, invCR1) as np.float32 scalars.

Sharding: pure data parallel — 96 (batch*channel) slices split 12 per core
across 8 NeuronCores; scalar partials are gathered and reduced on the host.

Per-slice device pipeline:
  - 5-level lifting transform: x-phase convs (along W, the free dim) as
    VectorE scalar_tensor_tensor chains; y-phase (along H, partitions) as
    TensorE matmuls against host-composed banded matrices A_l (yo2) / B_l
    (ye2) built from the runtime taps.
  - Subbands packed into a [128, 2048] staging tile (exact tetris, the deep
    sub-128-partition bands moved by SBUF-to-SBUF DMA).
  - Wrapped residuals, binning, and radix-16 one-hot masks (bf16, DVE 4x);
    joint (h,l) counts accumulated on TensorE into PSUM, dumped raw to DRAM.
  - Sum-of-squares partials via ScalarE Square+accum / DVE fused reduces.
"""

import os

import numpy as np
from contextlib import ExitStack

import concourse.bass as bass
import concourse.mybir as mybir
import concourse.tile as tile
from concourse import bacc
from concourse.bass_utils import run_bass_kernel_spmd

F32 = mybir.dt.float32
BF16 = mybir.dt.bfloat16
I32 = mybir.dt.int32
ALU = mybir.AluOpType
ACTF = mybir.ActivationFunctionType

N_CORES = 8
S0 = 512
NSL = 12            # slices per core (96 / 8)
STG = 2048          # staging free dim per slice (512*512/128)
RES = S0 * S0
FC = 512            # mask chunk width (free dim)
N_LEVELS = 5

# tap vector layout in the "tp" dram tensor (broadcast to [128, NT] on chip).
# The on-chip DVE y-phase needs uy, ry and negated py, cy.
TP_UY, TP_RY, TP_NPY, TP_NCY = 0, 3, 6, 9
NT = 12

# ---------------------------------------------------------------------------
# host-side y-phase matrix composition
# ---------------------------------------------------------------------------


def _make_x_mats(S, px, ux, cx, rx):
    """A (xo2T = A@curT) and B (xe2T = B@curT), composed in float64, cast f32.
    The x-lifting always runs all four steps at every level."""
    half = S // 2
    E = np.zeros((half, S))
    O = np.zeros((half, S))
    E[np.arange(half), 2 * np.arange(half)] = 1.0
    O[np.arange(half), 2 * np.arange(half) + 1] = 1.0

    def T(k):
        M = np.zeros((half, half))
        i = np.arange(half)
        M[i, i] = k[1]
        M[i[1:], i[1:] - 1] = k[0]
        M[i[:-1], i[:-1] + 1] = k[2]
        return M

    Xo1 = O - T(px.astype(np.float64)) @ E
    Xe1 = E + T(ux.astype(np.float64)) @ Xo1
    A = Xo1 - T(cx.astype(np.float64)) @ Xe1
    B = Xe1 + T(rx.astype(np.float64)) @ A
    return A.astype(np.float32), B.astype(np.float32)


def _y_block_structure():
    """Static nonzero-block structure of A/B per level: for each out-tile r,
    the in-tile col indices c whose [128,128] block is structurally nonzero
    (composed band halfwidth in the S domain is <= 7)."""
    plans = []
    for lvl in range(N_LEVELS):
        S = S0 >> lvl
        half = S // 2
        t_out = max(1, half // 128)
        t_in = max(1, S // 128)
        rows = []
        for r in range(t_out):
            m0 = 128 * r
            m1 = min(m0 + 128, half)
            j0 = max(0, 2 * m0 - 10)
            j1 = min(S - 1, 2 * (m1 - 1) + 10)
            cs = [c for c in range(t_in) if (128 * c <= j1 and 128 * c + 127 >= j0)]
            rows.append(cs)
        plans.append(rows)  # same structure for A and B
    return plans


_Y_PLANS = _y_block_structure()
NW = 2 * sum(len(cs) for plan in _Y_PLANS for cs in plan)


def _build_wx_host(px, ux, cx, rx):
    """Pack transposed [K, M] blocks of A/B into wx [NW, 128, 128] f32, in
    the exact emission order of the device builder."""
    wy = np.zeros((NW, 128, 128), np.float32)
    i = 0
    for lvl in range(N_LEVELS):
        S = S0 >> lvl
        half = S // 2
        A, B = _make_x_mats(S, px, ux, cx, rx)
        for M_ in (A, B):
            for r, cs in enumerate(_Y_PLANS[lvl]):
                m0 = 128 * r
                m1 = min(m0 + 128, half)
                for c in cs:
                    k0 = 128 * c
                    k1 = min(k0 + 128, S)
                    wy[i, : k1 - k0, : m1 - m0] = M_[m0:m1, k0:k1].T
                    i += 1
    assert i == NW, (i, NW)
    return wy


def _verify_block_coverage(px, ux, cx, rx):
    # every nonzero of A/B must land in a packed block
    for lvl in range(N_LEVELS):
        S = S0 >> lvl
        half = S // 2
        A, B = _make_x_mats(S, px, ux, cx, rx)
        for M_ in (A, B):
            mass = np.abs(M_).sum()
            cov = 0.0
            for r, cs in enumerate(_Y_PLANS[lvl]):
                m0, m1 = 128 * r, min(128 * r + 128, half)
                for c in cs:
                    k0, k1 = 128 * c, min(128 * c + 128, S)
                    cov += np.abs(M_[m0:m1, k0:k1]).sum()
            assert abs(cov - mass) < 1e-6 * max(mass, 1), (lvl, cov, mass)


# staging slots (transposed orientation) for the deep subbands: exact tetris
# of the final 128 columns. (p0, p1, c0, c1)
DEEP_SLOTS = {
    "l2xo2": (0, 64, 1920, 2048),
    "l2yo2": (64, 128, 1920, 1984),
    "l3xo2": (64, 96, 1984, 2048),
    "l3yo2": (96, 128, 1984, 2016),
    "l4xo2": (96, 112, 2016, 2048),
    "l4yo2": (112, 128, 2016, 2032),
    "ye4": (112, 128, 2032, 2048),
}

# ---------------------------------------------------------------------------
# device kernel
# ---------------------------------------------------------------------------


def _conv_step(nc, out_ap, base_ap, src_ap, tap_col, tp_sb, P, F):
    """out = base + sum_taps: center, then left, then right tap (tap signs are
    folded into the tp columns). Free-dim conv with zero padding."""
    k0 = tp_sb[0:P, tap_col : tap_col + 1]
    k1 = tp_sb[0:P, tap_col + 1 : tap_col + 2]
    k2 = tp_sb[0:P, tap_col + 2 : tap_col + 3]
    nc.vector.scalar_tensor_tensor(out_ap, src_ap, k1, base_ap, ALU.mult, ALU.add)
    nc.vector.scalar_tensor_tensor(
        out_ap[:, 1:F], src_ap[:, 0 : F - 1], k0, out_ap[:, 1:F], ALU.mult, ALU.add
    )
    nc.vector.scalar_tensor_tensor(
        out_ap[:, 0 : F - 1], src_ap[:, 1:F], k2, out_ap[:, 0 : F - 1], ALU.mult, ALU.add
    )


def _hist_pipeline(nc, ctx, pools, src_ap_full, kind, psum_ap, acc, acc_base):
    """Wrap/bin/mask/matmul pipeline over a [128, STG] f32 source.

    kind "delta": floor-mod wrap with invalid (v < -1) exclusion; acc slots
    acc_base + {0: sumsq(dm1), 1: s2, 2: n0}.
    kind "img": x in [0,1); acc slot acc_base + 0 = sumsq.

    floor(x) is built as rne(x) - (rne(x) > x) since the ISA only has
    round-to-nearest-even f32->int conversion (no mod/divide/trunc).
    Scratch tags A..K are reused across disjoint lifetimes to fit SBUF.
    """
    sc = pools["scratch"]
    mpool = pools["masks"]
    bias_m1 = pools["bias_m1"]
    bias_128 = pools["bias_128"]

    if kind == "delta":
        # g = fl(fl(v+1) * 0.5)  (= u/2 exactly, u := fl(v+1))
        g = sc.tile([128, STG], F32, tag="A")
        nc.vector.tensor_scalar(g[:], src_ap_full, 1.0, 0.5, ALU.add, ALU.mult)
        gi = sc.tile([128, STG], I32, tag="B")
        nc.scalar.activation(gi[:], g[:], ACTF.Identity)  # rne convert on ACT
        cg = sc.tile([128, STG], F32, tag="C")
        nc.vector.tensor_tensor(cg[:], gi[:], g[:], ALU.is_gt)
        ff = sc.tile([128, STG], F32, tag="D")
        nc.vector.tensor_tensor(ff[:], gi[:], cg[:], ALU.subtract)  # floor(g)
        frac = sc.tile([128, STG], F32, tag="E")
        nc.vector.tensor_tensor(frac[:], g[:], ff[:], ALU.subtract)  # in [0,1)
        cneg = sc.tile([128, STG], F32, tag="G")
        nc.vector.tensor_scalar(cneg[:], src_ap_full, -1.0, None, ALU.is_lt)
        # sumsq accumulates Square(2*frac - 1) directly (dm1 = fl(2*frac-1))
        junk_act = sc.tile([128, STG], F32, tag="A")
        nc.scalar.activation(
            junk_act[:], frac[:], ACTF.Square, bias=bias_m1[:, 0:1], scale=2.0,
            accum_out=acc[:, acc_base : acc_base + 1],
        )
        # t = 256*frac (vs ((dm1+1)*128): sub-ulp path difference can move a
        # borderline element one bin; ~1e-7 effect)
        t = sc.tile([128, STG], F32, tag="B")
        nc.scalar.activation(t[:], frac[:], ACTF.Identity, scale=256.0)
        tb = sc.tile([128, STG], F32, tag="C")
        nc.vector.scalar_tensor_tensor(tb[:], cneg[:], -512.0, t[:], ALU.mult, ALU.add)
        # s2 = sum over invalid of (8 - 4m) = (frac * -8 + 8) * cneg
        junk_dve = sc.tile([128, STG], F32, tag="B")
        nc.vector.affine_mul_reduce(
            junk_dve[:], acc[:, acc_base + 1 : acc_base + 2], frac[:], cneg[:], -8.0, 8.0
        )
        tsrc = tb
        n_h, h0, G = 16, 0, 8
    else:
        dm1 = sc.tile([128, STG], F32, tag="F")
        nc.vector.tensor_scalar(dm1[:], src_ap_full, 1.0, 1.0, ALU.add, ALU.subtract)
        junk_act = sc.tile([128, STG], F32, tag="A")
        nc.scalar.activation(
            junk_act[:], dm1[:], ACTF.Square, accum_out=acc[:, acc_base : acc_base + 1]
        )
        t = sc.tile([128, STG], F32, tag="B")
        nc.scalar.activation(t[:], dm1[:], ACTF.Identity, bias=bias_128[:, 0:1], scale=128.0)
        tsrc = t
        n_h, h0, G = 8, 8, 16

    # binf = floor(tsrc) via rne(tsrc - (0.5 - 2^-17)): the epsilon breaks
    # rne ties on exact-integer t (exact for img bins; for delta a sub-ulp
    # zone can shift ~2 counts/slice to an adjacent bin, ~1e-7 on entropy)
    bi2 = sc.tile([128, STG], I32, tag="E")
    nc.vector.tensor_scalar(bi2[:], tsrc[:], -0.49999237060546875, None, ALU.add)
    l_i = sc.tile([128, STG], I32, tag="F")
    nc.vector.tensor_scalar(l_i[:], bi2[:], 15, None, ALU.bitwise_and)
    h_i = sc.tile([128, STG], I32, tag="G")
    nc.vector.tensor_scalar(h_i[:], bi2[:], 4, None, ALU.arith_shift_right)
    lb = sc.tile([128, STG], BF16, tag="J")
    nc.scalar.activation(lb[:], l_i[:], ACTF.Identity)
    hb = sc.tile([128, STG], BF16, tag="K")
    nc.scalar.activation(hb[:], h_i[:], ACTF.Identity)

    n_chunks = STG // FC
    n_mm = FC // G
    for ch in range(n_chunks):
        c0 = ch * FC
        # layout [128, n_mm, bins*G]: sub-chunk g's operand is contiguous
        # (walrus requires a single free dim on matmul operands)
        mh = mpool.tile([128, n_mm, n_h * G], BF16, tag="mh")
        ml = mpool.tile([128, n_mm, 16 * G], BF16, tag="ml")
        if ABL["masks"]:
            for a in range(n_h):
                nc.vector.tensor_scalar(
                    mh[:, :, a * G : (a + 1) * G],
                    hb[:, c0 : c0 + FC],
                    float(h0 + a),
                    None,
                    ALU.is_equal,
                )
            for b in range(16):
                nc.vector.tensor_scalar(
                    ml[:, :, b * G : (b + 1) * G],
                    lb[:, c0 : c0 + FC],
                    float(b),
                    None,
                    ALU.is_equal,
                )
        else:
            nc.vector.memset(mh[:], 0.0)
            nc.vector.memset(ml[:], 0.0)
        if ABL["mm"]:
            for g_ in range(n_mm):
                nc.tensor.matmul(
                    psum_ap,
                    mh[:, g_, :],
                    ml[:, g_, :],
                    start=(ch == 0 and g_ == 0),
                    stop=(ch == n_chunks - 1 and g_ == n_mm - 1),
                    skip_group_check=True,
                )
        elif ch == 0:
            nc.tensor.matmul(
                psum_ap, mh[:, 0, :], ml[:, 0, :], start=True, stop=True,
                skip_group_check=True,
            )


ABL = {"masks": True, "mm": True, "prep": True, "x": True, "y": True, "hist_d": True, "hist_i": True}


def build_nc(nsl=NSL):
    nc = bacc.Bacc("TRN2", target_bir_lowering=False, debug=False)
    xs = nc.dram_tensor("xs", [nsl, S0, S0], F32, kind="ExternalInput")
    tp = nc.dram_tensor("tp", [NT], F32, kind="ExternalInput")
    wy = nc.dram_tensor("wy", [NW, 128, 128], F32, kind="ExternalInput")
    pd = nc.dram_tensor("pd", [nsl, 128, 128], F32, kind="ExternalOutput")
    pi = nc.dram_tensor("pi", [nsl, 128, 256], F32, kind="ExternalOutput")
    accd = nc.dram_tensor("accd", [128, nsl * 8], F32, kind="ExternalOutput")

    with tile.TileContext(nc) as tc:
        with ExitStack() as ctx:
            const = ctx.enter_context(tc.tile_pool(name="const", bufs=1))
            xpool = ctx.enter_context(tc.tile_pool(name="xpool", bufs=2))
            stgp = ctx.enter_context(tc.tile_pool(name="stgp", bufs=2))
            work = ctx.enter_context(tc.tile_pool(name="work", bufs=3))
            xe2p = ctx.enter_context(tc.tile_pool(name="xe2p", bufs=3))
            scratch = ctx.enter_context(tc.tile_pool(name="scratch", bufs=1))
            maskp = ctx.enter_context(tc.tile_pool(name="masks", bufs=2))
            psum = ctx.enter_context(tc.tile_pool(name="psum", bufs=2, space="PSUM"))
            ypsum = ctx.enter_context(tc.tile_pool(name="ypsum", bufs=4, space="PSUM"))
            bias_m1 = const.tile([128, 1], F32, tag="bias_m1")
            nc.vector.memset(bias_m1[:], -1.0)
            bias_128 = const.tile([128, 1], F32, tag="bias_128")
            nc.vector.memset(bias_128[:], 128.0)
            pools = {"scratch": scratch, "masks": maskp,
                     "bias_m1": bias_m1, "bias_128": bias_128}

            tp_sb = const.tile([128, NT], F32)
            nc.sync.dma_start(
                tp_sb[:], tp.ap().rearrange("(o n) -> o n", o=1).broadcast_to([128, NT])
            )
            wy_sb = const.tile([128, NW * 128], F32)
            nc.sync.dma_start(
                wy_sb[:].rearrange("k (n m) -> k n m", n=NW),
                wy.ap().rearrange("n k m -> k n m"),
            )
            acc = const.tile([128, nsl * 8], F32)
            nc.vector.memset(acc[:], 0.0)

            for s in range(nsl):
                # load slice: x_sb[p, t, w] = xs[s, 128t + p, w]
                x_sb = xpool.tile([128, 4, S0], F32, tag="x_sb")
                nc.sync.dma_start(
                    x_sb[:], xs.ap()[s].rearrange("(t p) w -> p t w", p=128)
                )
                x_flat = x_sb[:].rearrange("p t w -> p (t w)")

                stg = stgp.tile([128, STG], F32, tag="stg")

                cur_tiles = [x_sb[:, t, :] for t in range(4)]
                wy_idx = 0
                for lvl in range(N_LEVELS):
                    S = S0 >> lvl
                    half = S // 2
                    t_out = max(1, half // 128)

                    # ---- x-phase (PE): xo2T = A@curT (kind 0), xe2T = B@curT
                    xe2_tiles = []
                    for kind in range(2):
                        for r in range(t_out):
                            m0 = 128 * r
                            m1 = min(m0 + 128, half)
                            M = m1 - m0
                            cs = _Y_PLANS[lvl][r]
                            ps = ypsum.tile([M, S], F32, tag="yps")
                            for i, c in enumerate(cs):
                                K = min(128, S - 128 * c)
                                nc.tensor.matmul(
                                    ps[0:M, 0:S],
                                    wy_sb[0:K, 128 * wy_idx : 128 * wy_idx + M],
                                    cur_tiles[c][0:K, 0:S],
                                    start=(i == 0),
                                    stop=(i == len(cs) - 1),
                                    skip_group_check=True,
                                )
                                wy_idx += 1
                            if kind == 0:
                                if lvl == 0:
                                    nc.scalar.copy(
                                        stg[:, 512 * r : 512 * (r + 1)], ps[0:M, 0:S]
                                    )
                                elif lvl == 1:
                                    nc.scalar.copy(stg[:, 1536:1792], ps[0:M, 0:S])
                                elif lvl == 2:
                                    p0, p1, q0, q1 = DEEP_SLOTS["l2xo2"]
                                    nc.scalar.copy(stg[p0:p1, q0:q1], ps[0:M, 0:S])
                                else:
                                    key = "l3xo2" if lvl == 3 else "l4xo2"
                                    p0, p1, q0, q1 = DEEP_SLOTS[key]
                                    xo2s = work.tile([M, S], F32, tag=f"xo2s_{lvl}")
                                    nc.scalar.copy(xo2s[:], ps[0:M, 0:S])
                                    nc.sync.dma_start(stg[p0:p1, q0:q1], xo2s[:])
                            else:
                                xe2 = xe2p.tile([M, S], F32, tag=f"xe2_{lvl}")
                                nc.scalar.copy(xe2[:], ps[0:M, 0:S])
                                xe2_tiles.append((xe2, M))

                    # ---- y-phase (DVE) per xe2 tile
                    new_cur = []
                    for ti, (xe2, P) in enumerate(xe2_tiles):
                        ye_v = xe2[0:P, 0:S:2]
                        yo_v = xe2[0:P, 1:S:2]
                        yo1 = work.tile([P, half], F32, tag=f"yo1_{lvl}")
                        _conv_step(nc, yo1[:], yo_v, ye_v, TP_NPY, tp_sb, P, half)
                        if lvl < 2:
                            ye1 = work.tile([P, half], F32, tag=f"ye1_{lvl}")
                            _conv_step(nc, ye1[:], ye_v, yo1[:], TP_UY, tp_sb, P, half)
                            ye1_ap = ye1[:]
                        else:
                            ye1_ap = ye_v
                        if lvl == 0:
                            yo2 = stg[:, 1024 + 256 * ti : 1024 + 256 * (ti + 1)]
                        elif lvl == 1:
                            yo2 = stg[:, 1792:1920]
                        else:
                            yo2_t = work.tile([P, half], F32, tag=f"yo2_{lvl}")
                            yo2 = yo2_t[:]
                        _conv_step(nc, yo2, yo1[:], ye1_ap, TP_NCY, tp_sb, P, half)
                        if lvl >= 2:
                            key = {2: "l2yo2", 3: "l3yo2", 4: "l4yo2"}[lvl]
                            p0, p1, q0, q1 = DEEP_SLOTS[key]
                            nc.sync.dma_start(stg[p0:p1, q0:q1], yo2)
                        ye2 = work.tile([P, half], F32, tag=f"ye2_{lvl}")
                        _conv_step(nc, ye2[:], ye1_ap, yo2, TP_RY, tp_sb, P, half)
                        if lvl < N_LEVELS - 1:
                            new_cur.append(ye2[:])
                        else:
                            p0, p1, q0, q1 = DEEP_SLOTS["ye4"]
                            nc.sync.dma_start(stg[p0:p1, q0:q1], ye2[:])
                    cur_tiles = new_cur

                # ---- histograms + stats (img first: it only needs x_sb, so
                # its DVE mask work can fill the lifting ladder's bubbles)
                if ABL["hist_i"]:
                    ps_i = psum.tile([128, 256], F32, tag="ps_i")
                    _hist_pipeline(nc, ctx, pools, x_flat, "img", ps_i[:], acc, s * 8 + 4)
                    pi_sb = work.tile([128, 256], F32, tag="pi_sb")
                    nc.scalar.copy(pi_sb[:], ps_i[:])
                    nc.sync.dma_start(pi.ap()[s], pi_sb[:])

                if ABL["hist_d"]:
                    ps_d = psum.tile([128, 128], F32, tag="ps_d")
                    _hist_pipeline(nc, ctx, pools, stg[:], "delta", ps_d[:], acc, s * 8)
                    pd_sb = work.tile([128, 128], F32, tag="pd_sb")
                    nc.scalar.copy(pd_sb[:], ps_d[:])
                    nc.sync.dma_start(pd.ap()[s], pd_sb[:])

            nc.sync.dma_start(accd.ap()[:, :], acc[:])

    nc.compile()
    return nc


_NC_CACHE = {}


def _get_nc():
    if "nc" not in _NC_CACHE:
        _NC_CACHE["nc"] = build_nc()
    return _NC_CACHE["nc"]


LAST_INFO = {}


def kernel(x, px, ux, cx, rx, py, uy, cy, ry, _trace=False):
    x = np.ascontiguousarray(np.asarray(x, dtype=np.float32))
    px, ux, cx, rx, py, uy, cy, ry = (
        np.asarray(k, dtype=np.float32) for k in (px, ux, cx, rx, py, uy, cy, ry)
    )

    nc = _get_nc()

    tp_host = np.zeros(NT, np.float32)
    tp_host[TP_UY : TP_UY + 3] = uy
    tp_host[TP_RY : TP_RY + 3] = ry
    tp_host[TP_NPY : TP_NPY + 3] = -py
    tp_host[TP_NCY : TP_NCY + 3] = -cy
    wy_host = _build_wx_host(px, ux, cx, rx)

    # the device works on W-major (transposed) slices so the x-phase convs
    # run along partitions (TensorE) and the y-phase along the free dim
    shards = np.ascontiguousarray(
        x.reshape(N_CORES, NSL, S0, S0).transpose(0, 1, 3, 2)
    )
    in_maps = [
        {"xs": np.ascontiguousarray(shards[i]), "tp": tp_host, "wy": wy_host}
        for i in range(N_CORES)
    ]
    if not _trace:
        # the axon trace path needs antenv.axon_hooks, which this container
        # lacks; make sure an inherited BASS_TRACE can't route us there
        os.environ.setdefault("BASS_NEVER_TRACE", "1")
    res = run_bass_kernel_spmd(nc, in_maps, core_ids=list(range(N_CORES)), trace=_trace)
    LAST_INFO["exec_time_ns"] = res.exec_time_ns
    LAST_INFO["results"] = res

    counts_img = np.zeros((96, 256))
    counts_delta = np.zeros((96, 256))
    ss_img = np.zeros(96)
    ss_delta = np.zeros(96)
    for core in range(N_CORES):
        out = res.results[core]
        pd_ = out["pd"].astype(np.float64)
        pi_ = out["pi"].astype(np.float64)
        acc_ = out["accd"].astype(np.float64).sum(axis=0)
        for s in range(NSL):
            gs = core * NSL + s
            cd = np.einsum("afbf->ab", pd_[s].reshape(16, 8, 16, 8)).reshape(256)
            ci = np.einsum("afbf->ab", pi_[s].reshape(8, 16, 16, 16)).reshape(128)
            a = acc_[s * 8 : s * 8 + 8]
            sumsq_d, s2, n0, sumsq_i = a[0], a[1], a[2], a[4]
            cd[0] += n0
            counts_delta[gs] = cd
            counts_img[gs, 128:256] = ci
            ss_delta[gs] = sumsq_d + s2 - 8.0 * n0
            ss_img[gs] = sumsq_i

    loss1 = np.float32(255.0 * np.sqrt(ss_delta.sum() / (96 * RES)))
    loss0 = np.float32(255.0 * np.sqrt(ss_img.sum() / (96 * RES)))

    def ent(counts):
        p = counts / RES
        pz = np.where(p > 0, p, 1.0)
        return float(np.sum(-p * np.log2(pz)))

    invCR0 = np.float32(ent(counts_img) / (8.0 * 96))
    invCR1 = np.float32(ent(counts_delta) / (8.0 * 96))
    LAST_INFO.update(
        counts_img=counts_img, counts_delta=counts_delta, ss_img=ss_img, ss_delta=ss_delta
    )
    return loss1, loss0, invCR0, invCR1
